# revision 12
# baseline (speedup 1.0000x reference)
"""Trainium2 Bass kernel for the MembraneLayer problem.

Computation (per batch element b, per output neuron o):
    h[b, t, :] = inputs[b, t, :] @ w                       # (T, O)
    syn[b, 0] = mem[b, 0] = 0
    syn[b, t+1] = alpha * syn[b, t] + h[b, t]              # t = 0..T-2
    mem[b, t+1] = beta  * mem[b, t] + (1-beta) * syn[b, t]
Returns (syn_rec, mem_rec), each (B, T, O) float32.

Mapping: data-parallel over batch across 8 NeuronCores (16 batch rows per
core).  The host marshals inputs to (B, C, T) and outputs to (O, B, T) so
every DMA is a large fully-contiguous transfer (DMA descriptors are
per-partition contiguous runs; a transposing DMA would degenerate to
4-byte descriptors).  Per 4-batch "quad": 6 contraction-block DMA loads
(~1 MiB each), 24 accumulating fp32 matmuls (w stationary, contraction on
partitions) produce h^T = (O x T) tiles in PSUM, and the two first-order
recurrences run as DVE tensor_tensor_scan instructions (state =
data0*state + data1 along the free axis, one recurrence per partition).
The (1-beta)*syn cross term runs on the scalar engine (activation Copy
with per-partition scale).
"""

import numpy as np
from contextlib import ExitStack

import concourse.bacc as bacc
import concourse.bass as bass
import concourse.tile as tile
import concourse.mybir as mybir
from concourse import bass_utils

B, T, I, O = 128, 512, 700, 128
NCORES = 8
BS = B // NCORES            # batch rows per core (16)
G = 4                       # batch rows per quad (tile group)
NQ = BS // G                # quads per core (4)
KFULL = 5                   # full 128-row contraction blocks
KREM = I - KFULL * 128      # 60 remaining contraction rows
F32 = mybir.dt.float32

_CACHE = {}


def _build_nc():
    nc = bacc.Bacc("TRN2", target_bir_lowering=False, debug=False)

    # Host-marshalled layouts: x_t = inputs.transpose(0, 2, 1)  (BS, I, T)
    x_d = nc.dram_tensor("x", [BS, I, T], F32, kind="ExternalInput")
    w_d = nc.dram_tensor("w", [I, O], F32, kind="ExternalInput")
    a_bc_d = nc.dram_tensor("alpha_bc", [O, T], F32, kind="ExternalInput")
    b_bc_d = nc.dram_tensor("beta_bc", [O, T], F32, kind="ExternalInput")
    omb_d = nc.dram_tensor("omb", [O, 1], F32, kind="ExternalInput")
    # Outputs in (O, BS, T); host transposes back to (BS, T, O).
    syn_d = nc.dram_tensor("syn", [O, BS, T], F32, kind="ExternalOutput")
    mem_d = nc.dram_tensor("mem", [O, BS, T], F32, kind="ExternalOutput")

    mult = mybir.AluOpType.mult
    add = mybir.AluOpType.add

    with tile.TileContext(nc) as tc, ExitStack() as ctx:
        const_pool = ctx.enter_context(tc.tile_pool(name="const", bufs=1))
        x_pool = ctx.enter_context(tc.tile_pool(name="xin", bufs=8))
        x0_pool = ctx.enter_context(tc.tile_pool(name="x0", bufs=24))
        x0r_pool = ctx.enter_context(tc.tile_pool(name="x0r", bufs=4))
        xr_pool = ctx.enter_context(tc.tile_pool(name="xrem", bufs=2))
        psum_pool = ctx.enter_context(
            tc.tile_pool(name="hpsum", bufs=8, space=bass.MemorySpace.PSUM)
        )
        syn_pool = ctx.enter_context(tc.tile_pool(name="synout", bufs=6))
        mem_pool = ctx.enter_context(tc.tile_pool(name="memout", bufs=6))
        u_pool = ctx.enter_context(tc.tile_pool(name="u", bufs=3))

        # --- PE warm-up ---
        # The PE starts HAM-throttled (1.2 GHz) and takes ~3.4us of activity
        # to unthrottle.  Burn that window on dummy matmuls while the first
        # DMAs stream, so real matmuls run at 2.4 GHz from the start.
        warm_sb = const_pool.tile([128, 128], F32)
        nc.vector.memset(warm_sb[:, :], 0.0)
        warm_ps = psum_pool.tile([128, 32], F32, tag="ps")
        for _ in range(40):
            nc.tensor.matmul(
                warm_ps[:, :],
                warm_sb[:, :],
                warm_sb[:, 0:32],
                start=True,
                stop=True,
            )

        # --- constants ---
        # w_sb[p, k*O + o] = w[128k + p, o]   (contraction on partitions)
        w_sb = const_pool.tile([128, KFULL * O], F32)
        nc.sync.dma_start(
            w_sb[:, :].rearrange("p (k o) -> p k o", k=KFULL),
            w_d[0 : KFULL * 128, :].rearrange("(k p) o -> p k o", p=128),
        )
        w_rem = const_pool.tile([KREM, O], F32)
        nc.sync.dma_start(w_rem[:, :], w_d[KFULL * 128 : I, :])
        a_bc = const_pool.tile([128, T], F32)
        nc.sync.dma_start(a_bc[:, :], a_bc_d[:, :])
        b_bc = const_pool.tile([128, T], F32)
        nc.sync.dma_start(b_bc[:, :], b_bc_d[:, :])
        omb_sb = const_pool.tile([128, 1], F32)
        nc.sync.dma_start(omb_sb[:, :], omb_d[:, :])

        for q in range(NQ):
            b0 = q * G
            # Load all G batch rows of each contraction block in one DMA.
            # Alternate between the two HWDGE rings (SP and ACT) so
            # descriptor generation for loads runs in parallel.
            if q == 0:
                # First quad: per-batch-row tiles/DMAs (256KB pieces) so the
                # first matmuls start as soon as one (g, k) piece has landed.
                rhs = [[None] * (KFULL + 1) for _ in range(G)]
                for g in range(G):
                    for k in range(KFULL):
                        dma_eng = nc.sync if (k + g) % 2 == 0 else nc.scalar
                        xg = x0_pool.tile([128, T], F32)
                        dma_eng.dma_start(
                            xg[:, :], x_d[b0 + g, 128 * k : 128 * (k + 1), :]
                        )
                        rhs[g][k] = xg[:, :]
                    xgr = x0r_pool.tile([KREM, T], F32)
                    dma_eng = nc.sync if g % 2 == 0 else nc.scalar
                    dma_eng.dma_start(xgr[:, :], x_d[b0 + g, KFULL * 128 : I, :])
                    rhs[g][KFULL] = xgr[:, :]
            else:
                xks = []
                for k in range(KFULL):
                    dma_eng = nc.sync if k % 2 == 0 else nc.scalar
                    xq = x_pool.tile([128, G * T], F32)
                    dma_eng.dma_start(
                        xq[:, :].rearrange("p (g t) -> p g t", g=G),
                        x_d[b0 : b0 + G, 128 * k : 128 * (k + 1), :].rearrange(
                            "g c t -> c g t"
                        ),
                    )
                    xks.append(xq)
                xr = xr_pool.tile([KREM, G * T], F32)
                nc.scalar.dma_start(
                    xr[:, :].rearrange("p (g t) -> p g t", g=G),
                    x_d[b0 : b0 + G, KFULL * 128 : I, :].rearrange("g c t -> c g t"),
                )
                rhs = [
                    [xks[k][:, g * T : (g + 1) * T] for k in range(KFULL)]
                    + [xr[:, g * T : (g + 1) * T]]
                    for g in range(G)
                ]

            for g in range(G):
                # h^T for batch row b0+g: ps[o, t] = h[b0+g, t, o]
                ps = psum_pool.tile([128, T], F32, tag="ps")
                for k in range(KFULL):
                    nc.tensor.matmul(
                        ps[:, :],
                        w_sb[:, k * O : (k + 1) * O],
                        rhs[g][k],
                        start=(k == 0),
                        stop=False,
                    )
                nc.tensor.matmul(
                    ps[:, :], w_rem[:, :], rhs[g][KFULL], start=False, stop=True
                )

                # syn[:, t+1] = alpha*syn[:, t] + h[:, t]
                syn_t = syn_pool.tile([128, T], F32)
                nc.vector.memset(syn_t[:, 0:1], 0.0)
                nc.vector.tensor_tensor_scan(
                    syn_t[:, 1:T],
                    a_bc[:, 0 : T - 1],
                    ps[:, 0 : T - 1],
                    0.0,
                    mult,
                    add,
                )

                # u[:, t] = (1-beta)*syn[:, t] on the scalar engine
                u = u_pool.tile([128, T - 1], F32)
                nc.scalar.mul(u[:, :], syn_t[:, 0 : T - 1], omb_sb[:, :])

                # mem[:, t+1] = beta*mem[:, t] + u[:, t]
                mem_t = mem_pool.tile([128, T], F32)
                nc.vector.memset(mem_t[:, 0:1], 0.0)
                nc.vector.tensor_tensor_scan(
                    mem_t[:, 1:T],
                    b_bc[:, 0 : T - 1],
                    u[:, :],
                    0.0,
                    mult,
                    add,
                )

                # Store each batch row as soon as its scans finish; the
                # kernel tail then only waits on the last row's chain.
                nc.sync.dma_start(syn_d[:, b0 + g, :], syn_t[:, :])
                nc.scalar.dma_start(mem_d[:, b0 + g, :], mem_t[:, :])

    nc.compile()
    return nc


def get_nc():
    if "nc" not in _CACHE:
        _CACHE["nc"] = _build_nc()
    return _CACHE["nc"]


def make_in_maps(inputs, w, alpha, beta):
    x_t = np.ascontiguousarray(
        np.asarray(inputs, dtype=np.float32).transpose(0, 2, 1)
    )  # (B, I, T)
    w = np.ascontiguousarray(w, dtype=np.float32)
    alpha = np.asarray(alpha, dtype=np.float32).reshape(O)
    beta = np.asarray(beta, dtype=np.float32).reshape(O)
    a_bc = np.ascontiguousarray(np.broadcast_to(alpha[:, None], (O, T)))
    b_bc = np.ascontiguousarray(np.broadcast_to(beta[:, None], (O, T)))
    omb = np.ascontiguousarray((1.0 - beta)[:, None])
    return [
        {
            "x": x_t[i * BS : (i + 1) * BS],
            "w": w,
            "alpha_bc": a_bc,
            "beta_bc": b_bc,
            "omb": omb,
        }
        for i in range(NCORES)
    ]


def kernel(inputs, w, alpha, beta):
    nc = get_nc()
    in_maps = make_in_maps(inputs, w, alpha, beta)
    res = bass_utils.run_bass_kernel_spmd(nc, in_maps, list(range(NCORES))).results
    # Per-core outputs are (O, BS, T); gather over batch then -> (B, T, O).
    syn = np.concatenate([r["syn"] for r in res], axis=1).transpose(1, 2, 0)
    mem = np.concatenate([r["mem"] for r in res], axis=1).transpose(1, 2, 0)
    return np.ascontiguousarray(syn), np.ascontiguousarray(mem)


# revision 14
# speedup vs baseline: 1.0542x; 1.0542x over previous
"""Trainium2 Bass kernel for the MembraneLayer problem.

Computation (per batch element b, per output neuron o):
    h[b, t, :] = inputs[b, t, :] @ w                       # (T, O)
    syn[b, 0] = mem[b, 0] = 0
    syn[b, t+1] = alpha * syn[b, t] + h[b, t]              # t = 0..T-2
    mem[b, t+1] = beta  * mem[b, t] + (1-beta) * syn[b, t]
Returns (syn_rec, mem_rec), each (B, T, O) float32.

Mapping: data-parallel over batch across 8 NeuronCores (16 batch rows per
core).  The host marshals inputs to (B, C, T) and outputs to (O, B, T) so
every DMA is a large fully-contiguous transfer (DMA descriptors are
per-partition contiguous runs; a transposing DMA would degenerate to
4-byte descriptors).  Per 4-batch "quad": 6 contraction-block DMA loads
(~1 MiB each), 24 accumulating fp32 matmuls (w stationary, contraction on
partitions) produce h^T = (O x T) tiles in PSUM, and the two first-order
recurrences run as DVE tensor_tensor_scan instructions (state =
data0*state + data1 along the free axis, one recurrence per partition).
The (1-beta)*syn cross term runs on the scalar engine (activation Copy
with per-partition scale).
"""

import numpy as np
from contextlib import ExitStack

import concourse.bacc as bacc
import concourse.bass as bass
import concourse.tile as tile
import concourse.mybir as mybir
from concourse import bass_utils

B, T, I, O = 128, 512, 700, 128
NCORES = 8
BS = B // NCORES            # batch rows per core (16)
G = 4                       # batch rows per quad (tile group)
NQ = BS // G                # quads per core (4)
KFULL = 5                   # full 128-row contraction blocks
KREM = I - KFULL * 128      # 60 remaining contraction rows
F32 = mybir.dt.float32

_CACHE = {}


def _build_nc():
    nc = bacc.Bacc("TRN2", target_bir_lowering=False, debug=False)

    # Host-marshalled layouts: x_t = inputs.transpose(0, 2, 1)  (BS, I, T)
    x_d = nc.dram_tensor("x", [BS, I, T], F32, kind="ExternalInput")
    w_d = nc.dram_tensor("w", [I, O], F32, kind="ExternalInput")
    a_bc_d = nc.dram_tensor("alpha_bc", [O, T], F32, kind="ExternalInput")
    b_bc_d = nc.dram_tensor("beta_bc", [O, T], F32, kind="ExternalInput")
    omb_d = nc.dram_tensor("omb", [O, 1], F32, kind="ExternalInput")
    # Outputs in (O, BS, T); host transposes back to (BS, T, O).
    syn_d = nc.dram_tensor("syn", [O, BS, T], F32, kind="ExternalOutput")
    mem_d = nc.dram_tensor("mem", [O, BS, T], F32, kind="ExternalOutput")

    mult = mybir.AluOpType.mult
    add = mybir.AluOpType.add

    with tile.TileContext(nc) as tc, ExitStack() as ctx:
        const_pool = ctx.enter_context(tc.tile_pool(name="const", bufs=1))
        x_pool = ctx.enter_context(tc.tile_pool(name="xin", bufs=10))
        x0_pool = ctx.enter_context(tc.tile_pool(name="x0", bufs=12))
        x0r_pool = ctx.enter_context(tc.tile_pool(name="x0r", bufs=2))
        xr_pool = ctx.enter_context(tc.tile_pool(name="xrem", bufs=2))
        psum_pool = ctx.enter_context(
            tc.tile_pool(name="hpsum", bufs=8, space=bass.MemorySpace.PSUM)
        )
        syn_pool = ctx.enter_context(tc.tile_pool(name="synout", bufs=6))
        mem_pool = ctx.enter_context(tc.tile_pool(name="memout", bufs=6))
        u_pool = ctx.enter_context(tc.tile_pool(name="u", bufs=3))

        # --- constants ---
        # w_sb[p, k*O + o] = w[128k + p, o]   (contraction on partitions)
        w_sb = const_pool.tile([128, KFULL * O], F32)
        nc.sync.dma_start(
            w_sb[:, :].rearrange("p (k o) -> p k o", k=KFULL),
            w_d[0 : KFULL * 128, :].rearrange("(k p) o -> p k o", p=128),
        )
        w_rem = const_pool.tile([KREM, O], F32)
        nc.sync.dma_start(w_rem[:, :], w_d[KFULL * 128 : I, :])
        a_bc = const_pool.tile([128, T], F32)
        nc.sync.dma_start(a_bc[:, :], a_bc_d[:, :])
        b_bc = const_pool.tile([128, T], F32)
        nc.sync.dma_start(b_bc[:, :], b_bc_d[:, :])
        omb_sb = const_pool.tile([128, 1], F32)
        nc.sync.dma_start(omb_sb[:, :], omb_d[:, :])

        for q in range(NQ):
            b0 = q * G
            # Load all G batch rows of each contraction block in one DMA.
            # Alternate between the two HWDGE rings (SP and ACT) so
            # descriptor generation for loads runs in parallel.
            if q == 0:
                # First quad: per-batch-row tiles/DMAs (256KB pieces) so the
                # first matmuls start as soon as one (g, k) piece has landed.
                rhs = [[None] * (KFULL + 1) for _ in range(G)]
                for g in range(G):
                    for k in range(KFULL):
                        dma_eng = nc.sync if (k + g) % 2 == 0 else nc.scalar
                        xg = x0_pool.tile([128, T], F32)
                        dma_eng.dma_start(
                            xg[:, :], x_d[b0 + g, 128 * k : 128 * (k + 1), :]
                        )
                        rhs[g][k] = xg[:, :]
                    xgr = x0r_pool.tile([KREM, T], F32)
                    dma_eng = nc.sync if g % 2 == 0 else nc.scalar
                    dma_eng.dma_start(xgr[:, :], x_d[b0 + g, KFULL * 128 : I, :])
                    rhs[g][KFULL] = xgr[:, :]
            else:
                xks = []
                for k in range(KFULL):
                    dma_eng = nc.sync if k % 2 == 0 else nc.scalar
                    xq = x_pool.tile([128, G * T], F32)
                    dma_eng.dma_start(
                        xq[:, :].rearrange("p (g t) -> p g t", g=G),
                        x_d[b0 : b0 + G, 128 * k : 128 * (k + 1), :].rearrange(
                            "g c t -> c g t"
                        ),
                    )
                    xks.append(xq)
                xr = xr_pool.tile([KREM, G * T], F32)
                nc.scalar.dma_start(
                    xr[:, :].rearrange("p (g t) -> p g t", g=G),
                    x_d[b0 : b0 + G, KFULL * 128 : I, :].rearrange("g c t -> c g t"),
                )
                rhs = [
                    [xks[k][:, g * T : (g + 1) * T] for k in range(KFULL)]
                    + [xr[:, g * T : (g + 1) * T]]
                    for g in range(G)
                ]

            for g in range(G):
                # h^T for batch row b0+g: ps[o, t] = h[b0+g, t, o]
                ps = psum_pool.tile([128, T], F32, tag="ps")
                for k in range(KFULL):
                    nc.tensor.matmul(
                        ps[:, :],
                        w_sb[:, k * O : (k + 1) * O],
                        rhs[g][k],
                        start=(k == 0),
                        stop=False,
                    )
                nc.tensor.matmul(
                    ps[:, :], w_rem[:, :], rhs[g][KFULL], start=False, stop=True
                )

                # syn[:, t+1] = alpha*syn[:, t] + h[:, t]
                syn_t = syn_pool.tile([128, T], F32)
                nc.vector.memset(syn_t[:, 0:1], 0.0)
                nc.vector.tensor_tensor_scan(
                    syn_t[:, 1:T],
                    a_bc[:, 0 : T - 1],
                    ps[:, 0 : T - 1],
                    0.0,
                    mult,
                    add,
                )

                # u[:, t] = (1-beta)*syn[:, t] on the scalar engine
                u = u_pool.tile([128, T - 1], F32)
                nc.scalar.mul(u[:, :], syn_t[:, 0 : T - 1], omb_sb[:, :])

                # mem[:, t+1] = beta*mem[:, t] + u[:, t]
                mem_t = mem_pool.tile([128, T], F32)
                nc.vector.memset(mem_t[:, 0:1], 0.0)
                nc.vector.tensor_tensor_scan(
                    mem_t[:, 1:T],
                    b_bc[:, 0 : T - 1],
                    u[:, :],
                    0.0,
                    mult,
                    add,
                )

                # Store each batch row as soon as its scans finish; the
                # kernel tail then only waits on the last row's chain.
                nc.sync.dma_start(syn_d[:, b0 + g, :], syn_t[:, :])
                nc.scalar.dma_start(mem_d[:, b0 + g, :], mem_t[:, :])

    nc.compile()
    return nc


def get_nc():
    if "nc" not in _CACHE:
        _CACHE["nc"] = _build_nc()
    return _CACHE["nc"]


def make_in_maps(inputs, w, alpha, beta):
    x_t = np.ascontiguousarray(
        np.asarray(inputs, dtype=np.float32).transpose(0, 2, 1)
    )  # (B, I, T)
    w = np.ascontiguousarray(w, dtype=np.float32)
    alpha = np.asarray(alpha, dtype=np.float32).reshape(O)
    beta = np.asarray(beta, dtype=np.float32).reshape(O)
    a_bc = np.ascontiguousarray(np.broadcast_to(alpha[:, None], (O, T)))
    b_bc = np.ascontiguousarray(np.broadcast_to(beta[:, None], (O, T)))
    omb = np.ascontiguousarray((1.0 - beta)[:, None])
    return [
        {
            "x": x_t[i * BS : (i + 1) * BS],
            "w": w,
            "alpha_bc": a_bc,
            "beta_bc": b_bc,
            "omb": omb,
        }
        for i in range(NCORES)
    ]


def kernel(inputs, w, alpha, beta):
    nc = get_nc()
    in_maps = make_in_maps(inputs, w, alpha, beta)
    res = bass_utils.run_bass_kernel_spmd(nc, in_maps, list(range(NCORES))).results
    # Per-core outputs are (O, BS, T); gather over batch then -> (B, T, O).
    syn = np.concatenate([r["syn"] for r in res], axis=1).transpose(1, 2, 0)
    mem = np.concatenate([r["mem"] for r in res], axis=1).transpose(1, 2, 0)
    return np.ascontiguousarray(syn), np.ascontiguousarray(mem)


# revision 20
# speedup vs baseline: 1.0853x; 1.0295x over previous
"""Trainium2 Bass kernel for the MembraneLayer problem.

Computation (per batch element b, per output neuron o):
    h[b, t, :] = inputs[b, t, :] @ w                       # (T, O)
    syn[b, 0] = mem[b, 0] = 0
    syn[b, t+1] = alpha * syn[b, t] + h[b, t]              # t = 0..T-2
    mem[b, t+1] = beta  * mem[b, t] + (1-beta) * syn[b, t]
Returns (syn_rec, mem_rec), each (B, T, O) float32.

Mapping: data-parallel over batch across 8 NeuronCores (16 batch rows per
core).  The host marshals inputs to (B, C, T) and outputs to (O, B, T) so
every DMA is a large fully-contiguous transfer (DMA descriptors are
per-partition contiguous runs; a transposing DMA would degenerate to
4-byte descriptors).  Per 4-batch "quad": 6 contraction-block DMA loads
(~1 MiB each), 24 accumulating fp32 matmuls (w stationary, contraction on
partitions) produce h^T = (O x T) tiles in PSUM, and the two first-order
recurrences run as DVE tensor_tensor_scan instructions (state =
data0*state + data1 along the free axis, one recurrence per partition).
The (1-beta)*syn cross term runs on the scalar engine (activation Copy
with per-partition scale).
"""

import numpy as np
from contextlib import ExitStack

import concourse.bacc as bacc
import concourse.bass as bass
import concourse.tile as tile
import concourse.mybir as mybir
from concourse import bass_utils

B, T, I, O = 128, 512, 700, 128
NCORES = 8
BS = B // NCORES            # batch rows per core (16)
G = 4                       # batch rows per quad (tile group)
NQ = BS // G                # quads per core (4)
KFULL = 5                   # full 128-row contraction blocks
KREM = I - KFULL * 128      # 60 remaining contraction rows
F32 = mybir.dt.float32

_CACHE = {}


def _build_nc():
    nc = bacc.Bacc("TRN2", target_bir_lowering=False, debug=False)

    # Host-marshalled layouts: x_t = inputs.transpose(2, 0, 1)  (I, BS, T)
    # c-major so quad loads are 8KB-contiguous per partition.
    x_d = nc.dram_tensor("x", [I, BS, T], F32, kind="ExternalInput")
    w_d = nc.dram_tensor("w", [I, O], F32, kind="ExternalInput")
    a_bc_d = nc.dram_tensor("alpha_bc", [O, T], F32, kind="ExternalInput")
    b_bc_d = nc.dram_tensor("beta_bc", [O, T], F32, kind="ExternalInput")
    omb_d = nc.dram_tensor("omb", [O, 1], F32, kind="ExternalInput")
    # Outputs in (O, BS, T); host transposes back to (BS, T, O).
    syn_d = nc.dram_tensor("syn", [O, BS, T], F32, kind="ExternalOutput")
    mem_d = nc.dram_tensor("mem", [O, BS, T], F32, kind="ExternalOutput")

    mult = mybir.AluOpType.mult
    add = mybir.AluOpType.add

    with tile.TileContext(nc) as tc, ExitStack() as ctx:
        const_pool = ctx.enter_context(tc.tile_pool(name="const", bufs=1))
        x_pool = ctx.enter_context(tc.tile_pool(name="xin", bufs=10))
        x0_pool = ctx.enter_context(tc.tile_pool(name="x0", bufs=12))
        x0r_pool = ctx.enter_context(tc.tile_pool(name="x0r", bufs=2))
        xr_pool = ctx.enter_context(tc.tile_pool(name="xrem", bufs=2))
        psum_pool = ctx.enter_context(
            tc.tile_pool(name="hpsum", bufs=8, space=bass.MemorySpace.PSUM)
        )
        syn_pool = ctx.enter_context(tc.tile_pool(name="synout", bufs=6))
        mem_pool = ctx.enter_context(tc.tile_pool(name="memout", bufs=6))
        u_pool = ctx.enter_context(tc.tile_pool(name="u", bufs=3))

        # --- PE warm-up (bf16: ~150ns/matmul incl. FWL weight load) ---
        # The PE starts HAM-throttled (1.2 GHz) and needs ~3.4us of activity
        # to unthrottle; burn that window while the first DMAs stream.
        warm_sb = const_pool.tile([128, 128], mybir.dt.bfloat16)
        nc.vector.memset(warm_sb[:, :], 0.0)
        warm_ps = psum_pool.tile([128, 64], F32, tag="ps")
        for _ in range(30):
            nc.tensor.matmul(
                warm_ps[:, :],
                warm_sb[:, :],
                warm_sb[:, 0:64],
                start=True,
                stop=True,
            )

        # --- constants (SWDGE ring, so x loads lead the HWDGE rings) ---
        # w_sb[p, k*O + o] = w[128k + p, o]   (contraction on partitions)
        w_sb = const_pool.tile([128, KFULL * O], F32)
        nc.gpsimd.dma_start(
            w_sb[:, :].rearrange("p (k o) -> p k o", k=KFULL),
            w_d[0 : KFULL * 128, :].rearrange("(k p) o -> p k o", p=128),
        )
        w_rem = const_pool.tile([KREM, O], F32)
        nc.gpsimd.dma_start(w_rem[:, :], w_d[KFULL * 128 : I, :])
        a_bc = const_pool.tile([128, T], F32)
        nc.gpsimd.dma_start(a_bc[:, :], a_bc_d[:, :])
        b_bc = const_pool.tile([128, T], F32)
        nc.gpsimd.dma_start(b_bc[:, :], b_bc_d[:, :])
        omb_sb = const_pool.tile([128, 1], F32)
        nc.gpsimd.dma_start(omb_sb[:, :], omb_d[:, :])

        for q in range(NQ):
            b0 = q * G
            # Load all G batch rows of each contraction block in one DMA.
            # Alternate between the two HWDGE rings (SP and ACT) so
            # descriptor generation for loads runs in parallel.
            if q == 0:
                # First quad: per-batch-row tiles/DMAs (256KB pieces) so the
                # first matmuls start as soon as one (g, k) piece has landed.
                rhs = [[None] * (KFULL + 1) for _ in range(G)]
                for g in range(G):
                    for k in range(KFULL):
                        dma_eng = nc.sync if (k + g) % 2 == 0 else nc.scalar
                        xg = x0_pool.tile([128, T], F32)
                        dma_eng.dma_start(
                            xg[:, :], x_d[128 * k : 128 * (k + 1), b0 + g, :]
                        )
                        rhs[g][k] = xg[:, :]
                    xgr = x0r_pool.tile([KREM, T], F32)
                    dma_eng = nc.sync if g % 2 == 0 else nc.scalar
                    dma_eng.dma_start(xgr[:, :], x_d[KFULL * 128 : I, b0 + g, :])
                    rhs[g][KFULL] = xgr[:, :]
            else:
                xks = []
                for k in range(KFULL):
                    dma_eng = nc.sync if k % 2 == 0 else nc.scalar
                    xq = x_pool.tile([128, G * T], F32)
                    dma_eng.dma_start(
                        xq[:, :].rearrange("p (g t) -> p g t", g=G),
                        x_d[128 * k : 128 * (k + 1), b0 : b0 + G, :],
                    )
                    xks.append(xq)
                xr = xr_pool.tile([KREM, G * T], F32)
                nc.scalar.dma_start(
                    xr[:, :].rearrange("p (g t) -> p g t", g=G),
                    x_d[KFULL * 128 : I, b0 : b0 + G, :],
                )
                rhs = [
                    [xks[k][:, g * T : (g + 1) * T] for k in range(KFULL)]
                    + [xr[:, g * T : (g + 1) * T]]
                    for g in range(G)
                ]

            for g in range(G):
                # h^T for batch row b0+g: ps[o, t] = h[b0+g, t, o]
                ps = psum_pool.tile([128, T], F32, tag="ps")
                for k in range(KFULL):
                    nc.tensor.matmul(
                        ps[:, :],
                        w_sb[:, k * O : (k + 1) * O],
                        rhs[g][k],
                        start=(k == 0),
                        stop=False,
                    )
                nc.tensor.matmul(
                    ps[:, :], w_rem[:, :], rhs[g][KFULL], start=False, stop=True
                )

                # syn[:, t+1] = alpha*syn[:, t] + h[:, t]
                syn_t = syn_pool.tile([128, T], F32)
                nc.vector.memset(syn_t[:, 0:1], 0.0)
                nc.vector.tensor_tensor_scan(
                    syn_t[:, 1:T],
                    a_bc[:, 0 : T - 1],
                    ps[:, 0 : T - 1],
                    0.0,
                    mult,
                    add,
                )

                # u[:, t] = (1-beta)*syn[:, t] on the scalar engine
                u = u_pool.tile([128, T - 1], F32)
                nc.scalar.mul(u[:, :], syn_t[:, 0 : T - 1], omb_sb[:, :])

                # mem[:, t+1] = beta*mem[:, t] + u[:, t]
                mem_t = mem_pool.tile([128, T], F32)
                nc.vector.memset(mem_t[:, 0:1], 0.0)
                nc.vector.tensor_tensor_scan(
                    mem_t[:, 1:T],
                    b_bc[:, 0 : T - 1],
                    u[:, :],
                    0.0,
                    mult,
                    add,
                )

                # Store each batch row as soon as its scans finish; the
                # kernel tail then only waits on the last row's chain.
                nc.sync.dma_start(syn_d[:, b0 + g, :], syn_t[:, :])
                nc.scalar.dma_start(mem_d[:, b0 + g, :], mem_t[:, :])

    nc.compile()
    return nc


def get_nc():
    if "nc" not in _CACHE:
        _CACHE["nc"] = _build_nc()
    return _CACHE["nc"]


def make_in_maps(inputs, w, alpha, beta):
    x_t = np.asarray(inputs, dtype=np.float32).transpose(2, 0, 1)  # (I, B, T) view
    w = np.ascontiguousarray(w, dtype=np.float32)
    alpha = np.asarray(alpha, dtype=np.float32).reshape(O)
    beta = np.asarray(beta, dtype=np.float32).reshape(O)
    a_bc = np.ascontiguousarray(np.broadcast_to(alpha[:, None], (O, T)))
    b_bc = np.ascontiguousarray(np.broadcast_to(beta[:, None], (O, T)))
    omb = np.ascontiguousarray((1.0 - beta)[:, None])
    return [
        {
            "x": np.ascontiguousarray(x_t[:, i * BS : (i + 1) * BS, :]),
            "w": w,
            "alpha_bc": a_bc,
            "beta_bc": b_bc,
            "omb": omb,
        }
        for i in range(NCORES)
    ]


def kernel(inputs, w, alpha, beta):
    nc = get_nc()
    in_maps = make_in_maps(inputs, w, alpha, beta)
    res = bass_utils.run_bass_kernel_spmd(nc, in_maps, list(range(NCORES))).results
    # Per-core outputs are (O, BS, T); gather over batch then -> (B, T, O).
    syn = np.concatenate([r["syn"] for r in res], axis=1).transpose(1, 2, 0)
    mem = np.concatenate([r["mem"] for r in res], axis=1).transpose(1, 2, 0)
    return np.ascontiguousarray(syn), np.ascontiguousarray(mem)


# revision 24
# speedup vs baseline: 1.1374x; 1.0480x over previous
"""Trainium2 Bass kernel for the MembraneLayer problem.

Computation (per batch element b, per output neuron o):
    h[b, t, :] = inputs[b, t, :] @ w                       # (T, O)
    syn[b, 0] = mem[b, 0] = 0
    syn[b, t+1] = alpha * syn[b, t] + h[b, t]              # t = 0..T-2
    mem[b, t+1] = beta  * mem[b, t] + (1-beta) * syn[b, t]
Returns (syn_rec, mem_rec), each (B, T, O) float32.

Mapping: data-parallel over batch across 8 NeuronCores (16 batch rows per
core).  The host marshals inputs to (B, C, T) and outputs to (O, B, T) so
every DMA is a large fully-contiguous transfer (DMA descriptors are
per-partition contiguous runs; a transposing DMA would degenerate to
4-byte descriptors).  Per 4-batch "quad": 6 contraction-block DMA loads
(~1 MiB each), 24 accumulating fp32 matmuls (w stationary, contraction on
partitions) produce h^T = (O x T) tiles in PSUM, and the two first-order
recurrences run as DVE tensor_tensor_scan instructions (state =
data0*state + data1 along the free axis, one recurrence per partition).
The (1-beta)*syn cross term runs on the scalar engine (activation Copy
with per-partition scale).
"""

import numpy as np
from contextlib import ExitStack

import concourse.bacc as bacc
import concourse.bass as bass
import concourse.tile as tile
import concourse.mybir as mybir
from concourse import bass_utils

B, T, I, O = 128, 512, 700, 128
NCORES = 8
BS = B // NCORES            # batch rows per core (16)
G = 4                       # batch rows per quad (tile group)
NQ = BS // G                # quads per core (4)
KFULL = 5                   # full 128-row contraction blocks
KREM = I - KFULL * 128      # 60 remaining contraction rows
F32 = mybir.dt.float32

_CACHE = {}


def _build_nc():
    nc = bacc.Bacc("TRN2", target_bir_lowering=False, debug=False)

    # Host-marshalled layouts: x_t = inputs.transpose(2, 0, 1)  (I, BS, T)
    # c-major so quad loads are 8KB-contiguous per partition.
    x_d = nc.dram_tensor("x", [I, BS, T], F32, kind="ExternalInput")
    w_d = nc.dram_tensor("w", [I, O], F32, kind="ExternalInput")
    a_bc_d = nc.dram_tensor("alpha_bc", [O, T], F32, kind="ExternalInput")
    b_bc_d = nc.dram_tensor("beta_bc", [O, T], F32, kind="ExternalInput")
    omb_d = nc.dram_tensor("omb", [O, 1], F32, kind="ExternalInput")
    # Outputs in (O, BS, T); host transposes back to (BS, T, O).
    syn_d = nc.dram_tensor("syn", [O, BS, T], F32, kind="ExternalOutput")
    mem_d = nc.dram_tensor("mem", [O, BS, T], F32, kind="ExternalOutput")

    mult = mybir.AluOpType.mult
    add = mybir.AluOpType.add

    with tile.TileContext(nc) as tc, ExitStack() as ctx:
        const_pool = ctx.enter_context(tc.tile_pool(name="const", bufs=1))
        x_pool = ctx.enter_context(tc.tile_pool(name="xin", bufs=10))
        x0_pool = ctx.enter_context(tc.tile_pool(name="x0", bufs=5))
        x0r_pool = ctx.enter_context(tc.tile_pool(name="x0r", bufs=1))
        xr_pool = ctx.enter_context(tc.tile_pool(name="xrem", bufs=2))
        psum_pool = ctx.enter_context(
            tc.tile_pool(name="hpsum", bufs=8, space=bass.MemorySpace.PSUM)
        )
        syn_pool = ctx.enter_context(tc.tile_pool(name="synout", bufs=6))
        mem_pool = ctx.enter_context(tc.tile_pool(name="memout", bufs=6))
        u_pool = ctx.enter_context(tc.tile_pool(name="u", bufs=3))

        # --- PE warm-up (bf16, ~60ns/matmul) ---
        # The PE starts HAM-throttled (1.2 GHz) and needs ~3.4us of activity
        # to unthrottle.  Keep it continuously busy from kernel start until
        # the first real operand lands (~10us): no memset dependency (the
        # operand values are irrelevant; the PSUM bank is discarded).
        warm_sb = const_pool.tile([128, 128], mybir.dt.bfloat16)
        nc.gpsimd.memset(warm_sb[:, :], 0.0)
        warm_ps = psum_pool.tile([128, 64], F32, tag="ps")
        for _ in range(100):
            nc.tensor.matmul(
                warm_ps[:, :],
                warm_sb[:, :],
                warm_sb[:, 0:64],
                start=True,
                stop=True,
            )

        # --- constants (SWDGE ring, so x loads lead the HWDGE rings) ---
        # w_sb[p, k*O + o] = w[128k + p, o]   (contraction on partitions)
        w_sb = const_pool.tile([128, KFULL * O], F32)
        nc.gpsimd.dma_start(
            w_sb[:, :].rearrange("p (k o) -> p k o", k=KFULL),
            w_d[0 : KFULL * 128, :].rearrange("(k p) o -> p k o", p=128),
        )
        w_rem = const_pool.tile([KREM, O], F32)
        nc.gpsimd.dma_start(w_rem[:, :], w_d[KFULL * 128 : I, :])
        a_bc = const_pool.tile([128, T], F32)
        nc.gpsimd.dma_start(a_bc[:, :], a_bc_d[:, :])
        b_bc = const_pool.tile([128, T], F32)
        nc.gpsimd.dma_start(b_bc[:, :], b_bc_d[:, :])
        omb_sb = const_pool.tile([128, 1], F32)
        nc.gpsimd.dma_start(omb_sb[:, :], omb_d[:, :])

        for q in range(NQ):
            b0 = q * G
            # Load all G batch rows of each contraction block in one DMA.
            # Alternate between the two HWDGE rings (SP and ACT) so
            # descriptor generation for loads runs in parallel.
            if q == 0:
                # Redundant per-piece loads of just the FIRST batch row (it
                # re-arrives inside the quad tiles below): 256KB pieces land
                # first, so real matmuls start ~7us earlier.
                b0_rhs = []
                for k in range(KFULL):
                    dma_eng = nc.sync if k % 2 == 0 else nc.scalar
                    xg = x0_pool.tile([128, T], F32)
                    dma_eng.dma_start(xg[:, :], x_d[128 * k : 128 * (k + 1), 0, :])
                    b0_rhs.append(xg[:, :])
                xgr = x0r_pool.tile([KREM, T], F32)
                nc.scalar.dma_start(xgr[:, :], x_d[KFULL * 128 : I, 0, :])
                b0_rhs.append(xgr[:, :])

            xks = []
            for k in range(KFULL):
                dma_eng = nc.sync if k % 2 == 0 else nc.scalar
                xq = x_pool.tile([128, G * T], F32)
                dma_eng.dma_start(
                    xq[:, :].rearrange("p (g t) -> p g t", g=G),
                    x_d[128 * k : 128 * (k + 1), b0 : b0 + G, :],
                )
                xks.append(xq)
            xr = xr_pool.tile([KREM, G * T], F32)
            nc.scalar.dma_start(
                xr[:, :].rearrange("p (g t) -> p g t", g=G),
                x_d[KFULL * 128 : I, b0 : b0 + G, :],
            )
            rhs = [
                [xks[k][:, g * T : (g + 1) * T] for k in range(KFULL)]
                + [xr[:, g * T : (g + 1) * T]]
                for g in range(G)
            ]
            if q == 0:
                rhs[0] = b0_rhs

            for g in range(G):
                # h^T for batch row b0+g: ps[o, t] = h[b0+g, t, o]
                ps = psum_pool.tile([128, T], F32, tag="ps")
                for k in range(KFULL):
                    nc.tensor.matmul(
                        ps[:, :],
                        w_sb[:, k * O : (k + 1) * O],
                        rhs[g][k],
                        start=(k == 0),
                        stop=False,
                    )
                nc.tensor.matmul(
                    ps[:, :], w_rem[:, :], rhs[g][KFULL], start=False, stop=True
                )

                # syn[:, t+1] = alpha*syn[:, t] + h[:, t]
                syn_t = syn_pool.tile([128, T], F32)
                nc.vector.memset(syn_t[:, 0:1], 0.0)
                nc.vector.tensor_tensor_scan(
                    syn_t[:, 1:T],
                    a_bc[:, 0 : T - 1],
                    ps[:, 0 : T - 1],
                    0.0,
                    mult,
                    add,
                )

                # u[:, t] = (1-beta)*syn[:, t] on the scalar engine
                u = u_pool.tile([128, T - 1], F32)
                nc.scalar.mul(u[:, :], syn_t[:, 0 : T - 1], omb_sb[:, :])

                # mem[:, t+1] = beta*mem[:, t] + u[:, t]
                mem_t = mem_pool.tile([128, T], F32)
                nc.vector.memset(mem_t[:, 0:1], 0.0)
                nc.vector.tensor_tensor_scan(
                    mem_t[:, 1:T],
                    b_bc[:, 0 : T - 1],
                    u[:, :],
                    0.0,
                    mult,
                    add,
                )

                # Store each batch row as soon as its scans finish; the
                # kernel tail then only waits on the last row's chain.
                nc.sync.dma_start(syn_d[:, b0 + g, :], syn_t[:, :])
                nc.scalar.dma_start(mem_d[:, b0 + g, :], mem_t[:, :])

    nc.compile()
    return nc


def get_nc():
    if "nc" not in _CACHE:
        _CACHE["nc"] = _build_nc()
    return _CACHE["nc"]


def make_in_maps(inputs, w, alpha, beta):
    x_t = np.asarray(inputs, dtype=np.float32).transpose(2, 0, 1)  # (I, B, T) view
    w = np.ascontiguousarray(w, dtype=np.float32)
    alpha = np.asarray(alpha, dtype=np.float32).reshape(O)
    beta = np.asarray(beta, dtype=np.float32).reshape(O)
    a_bc = np.ascontiguousarray(np.broadcast_to(alpha[:, None], (O, T)))
    b_bc = np.ascontiguousarray(np.broadcast_to(beta[:, None], (O, T)))
    omb = np.ascontiguousarray((1.0 - beta)[:, None])
    return [
        {
            "x": np.ascontiguousarray(x_t[:, i * BS : (i + 1) * BS, :]),
            "w": w,
            "alpha_bc": a_bc,
            "beta_bc": b_bc,
            "omb": omb,
        }
        for i in range(NCORES)
    ]


def kernel(inputs, w, alpha, beta):
    nc = get_nc()
    in_maps = make_in_maps(inputs, w, alpha, beta)
    res = bass_utils.run_bass_kernel_spmd(nc, in_maps, list(range(NCORES))).results
    # Per-core outputs are (O, BS, T); gather over batch then -> (B, T, O).
    syn = np.concatenate([r["syn"] for r in res], axis=1).transpose(1, 2, 0)
    mem = np.concatenate([r["mem"] for r in res], axis=1).transpose(1, 2, 0)
    return np.ascontiguousarray(syn), np.ascontiguousarray(mem)
